# revision 4
# baseline (speedup 1.0000x reference)
"""Entmax-bisect (alpha-entmax via 10-step bisection) on Trainium2.

Data-parallel over 8 NeuronCores: X [8, 2048, 4096] is sharded on the
leading dim (2048 rows x 4096 per core); the reduction dim stays local.
alpha is a replicated scalar folded into compile-time constants.

Math (per row, in the tau-hat = tau/am1 domain, exact for am1 = 2^j):
    relu((Xs - tau))^2 * am1^-2-invariance => iterate on y = X directly:
    mx = max(x); tlo0 = mx - 1/am1; dm0 = 1/am1 - (1/d)^am1/am1 (const)
    pass k: s = sum relu(x - (tlo + dm0*2^-k))^2 ; tlo += dm0*2^-k if s>=thr
    thr = am1^-2;  p = relu(x - tau10)^2 ; out = p / sum(p)

Engine split per [128,4096] tile (measured ns):
    CASTMAX (DVE custom 1x, 4418): x f32 -> x16 fp16 + row max
    ACT-lane pass:  DVE TS relu fp16 4x (1339) + ACT Square+accum (3986)
    DVE-lane pass:  fused custom relu^2+accum 1x (4477)
    normalize:      GPSIMD normalize_recip (3627)  [p and out in f32]
    per-row tau updates: tiny [128,1] custom DVE ops (~140 each)
"""

import math
from operator import add as _op_add

import numpy as np

import concourse.bass as bass  # noqa: F401
import concourse.tile as tile
from concourse import bacc, mybir
from concourse.bass_utils import run_bass_kernel_spmd

N_CORES = 8
D = 4096
N_ITER = 10
P = 128

DVE_THR = 5      # passes with ((k-1)*4 + PH*t) % 20 < DVE_THR run on DVE lane
PH = 7           # per-tile rotation of the lane pattern
OFF_STRIDE = 2   # pipeline skew between consecutive tiles

TRACE = False
LAST_RESULT = None

_NC_CACHE = {}


# ---------- runtime registration of custom DVE ops ----------------------

def _register_dve_op(op_name, spec):
    from concourse import dve_ops as DO
    from concourse.dve_spec import lower, _has_src1 as has_src1
    from concourse.dve_uop import DveOpSpec

    for o in DO.OPS:
        if o.name == op_name:
            return o
    row = DO._CUSTOM_DVE_ROW_BASE + len(DO.OPS)
    assert row < 0x20
    shas = {}
    for ver in ("v3", "v4"):
        s = DveOpSpec(name=op_name, opcode=row, uops=lower(spec, ver=ver),
                      rd1_en=has_src1(spec))
        shas[ver] = s.sha(ver)
    op = DO.DveOp(op_name, spec, subdim=False, uops_sha=shas)
    DO.OPS.append(op)
    DO._SUB_OPCODE_FOR_NAME[op_name] = row
    DO.CUSTOM_DVE_SPECS[op_name] = spec
    return op


def _get_ops():
    from concourse.dve_spec import (
        Spec, Src0, Src1, C0, C1, C2, Zero, relu, select, sq, maxx,
    )

    def _ref_step2(in0, in1, c0, c1, c2):
        b = np.maximum(in0.astype(np.float32) - c0 - c2, 0.0) ** 2
        b = b.astype(np.float32)
        return b, c1 + b.reshape(b.shape[0], -1).sum(axis=-1, keepdims=True)

    # out = relu(x - (s0 + imm2))^2 ; accum = s1 + sum(out)
    step2 = _register_dve_op(
        "ENTMAX_STEP2_ANT",
        Spec(body=sq(relu((Src0 - C0) - C2)), accum=_op_add, accum_init=C1,
             reference=_ref_step2),
    )

    def _ref_castmax(in0, in1, c0, c1, c2):
        b = in0.astype(np.float32) + 0.0
        return b, np.max(b, axis=-1, keepdims=True)

    # out = cast(x) ; accum = row max
    castmax = _register_dve_op(
        "ENTMAX_CASTMAX_ANT",
        Spec(body=Src0 + Zero, accum=maxx, reference=_ref_castmax),
    )

    # out = select(acc >= 0, tlo + s0, tlo)   (DVE-lane tau update)
    updd = _register_dve_op(
        "ENTMAX_UPDD_ANT",
        Spec(body=select(Src0 >= Zero, Src1 + C0, Src1),
             reference=lambda in0, in1, s0, s1, imm2: np.where(
                 in0 >= 0, in1 + s0, in1).astype(np.float32)),
    )
    # out = select((acc + imm2) >= 0, tau, tlo)  (ACT-lane tau update;
    # s0 = tau tile, s1 = tlo tile)
    upd = _register_dve_op(
        "ENTMAX_TAU_UPD_ANT",
        Spec(body=select((Src0 + C2) * C0 >= Zero, Src1, C1),
             reference=lambda in0, in1, s0, s1, imm2: np.where(
                 (in0 + imm2) * s0 >= 0, in1, s1).astype(np.float32)),
    )
    return step2, castmax, updd, upd


def _is_dve_pass(t, k):
    return ((k - 1) * 4 + PH * t) % 20 < DVE_THR


def _build(am1: float, rows: int):
    """Build the single-core Bass program for a [rows, D] shard."""
    f32 = mybir.dt.float32
    f16 = mybir.dt.float16
    AF = mybir.ActivationFunctionType
    OP = mybir.AluOpType
    STEP2, CASTMAX, UPDD, UPD = _get_ops()

    c_lo = 1.0 / am1
    pw = float(np.power(np.float32(1.0 / D), np.float32(am1)))
    c_hi = pw / am1
    dm0 = c_lo - c_hi          # bracket width: a compile-time constant
    thr = 1.0 / (am1 * am1)    # sum relu^2 decision threshold

    nc = bacc.Bacc(None, target_bir_lowering=False)
    Xd = nc.declare_dram_parameter("X", [rows, D], f32, isOutput=False)
    Od = nc.declare_dram_parameter("OUT", [rows, D], f32, isOutput=True)
    ntiles = rows // P
    HC2 = D // 2

    with tile.TileContext(nc) as tc:
        with (
            tc.tile_pool(name="xp", bufs=2) as xp,       # f32 staging
            tc.tile_pool(name="x16p", bufs=6) as x16p,   # fp16 working set
            tc.tile_pool(name="pp", bufs=2) as pp,       # f32 p (last pass)
            tc.tile_pool(name="op", bufs=2) as outp,     # f32 normalized out
            tc.tile_pool(name="sc", bufs=2) as scp,      # rotating scratch
            tc.tile_pool(name="st", bufs=16) as st,
        ):
            xt, x16, mx, tlo, pf, ssum = {}, {}, {}, {}, {}, {}
            # pass outputs for k<10 are never read; DVE-lane shares one
            # scratch (engine-serial), ACT-lane rotates 2 so prep(n+1)
            # overlaps Square(n)
            scr_v = scp.tile([P, D], f16, tag="scrv", name="scrv")

            def emit_dma(t):
                xt[t] = xp.tile([P, D], f32, tag="xt", name="xt")
                if t == 0:
                    nc.sync.dma_start(out=xt[t][:, :HC2],
                                      in_=Xd[t * P:(t + 1) * P, :HC2])
                    nc.sync.dma_start(out=xt[t][:, HC2:],
                                      in_=Xd[t * P:(t + 1) * P, HC2:])
                else:
                    nc.sync.dma_start(out=xt[t][:],
                                      in_=Xd[t * P:(t + 1) * P, :])

            def emit_setup(t):
                x16[t] = x16p.tile([P, D], f16, tag="x16", name="x16")
                mx[t] = st.tile([P, 1], f32, tag="mx", name="mx")
                nc.vector._custom_dve(CASTMAX, out=x16[t][:], in0=xt[t][:],
                                      accum_out=mx[t][:])
                tlo[t] = st.tile([P, 1], f32, tag="tlo", name="tlo")
                nc.vector.tensor_scalar(tlo[t][:], mx[t][:], c_lo, None,
                                        OP.subtract)

            def emit_pass(t, k):
                last = k == N_ITER
                ck = dm0 * (0.5 ** k)
                if last:
                    pf[t] = pp.tile([P, D], f32, tag="pf", name="pf")
                acc = st.tile([P, 1], f32, tag="acc", name="acc")
                if _is_dve_pass(t, k):
                    # fused DVE lane: relu^2 + accum in one 1x op
                    nc.vector._custom_dve(
                        STEP2, out=(pf[t][:] if last else scr_v[:]),
                        in0=x16[t][:], s0=tlo[t][:],
                        s1=(0.0 if last else -thr), imm2=ck,
                        accum_out=acc[:])
                    if not last:
                        tlo_new = st.tile([P, 1], f32, tag="tlo", name="tlo")
                        nc.vector._custom_dve(
                            UPDD, out=tlo_new[:], in0=acc[:], in1=tlo[t][:],
                            s0=ck, s1=0.0, imm2=0.0)
                        tlo[t] = tlo_new
                    else:
                        ssum[t] = acc
                else:
                    # ACT lane: DVE 4x relu prep + ACT Square+accum
                    tau = st.tile([P, 1], f32, tag="tau", name="tau")
                    nc.vector.tensor_scalar(tau[:], tlo[t][:], ck, None,
                                            OP.add)
                    ra = scp.tile([P, D], f16, tag="scra", name="scra")
                    nc.vector.tensor_scalar(ra[:], x16[t][:], tau[:],
                                            tau[:], OP.max, OP.subtract)
                    nc.scalar.activation(pf[t][:] if last else ra[:],
                                         ra[:], AF.Square, bias=0.0,
                                         scale=1.0, accum_out=acc[:])
                    if not last:
                        tlo_new = st.tile([P, 1], f32, tag="tlo", name="tlo")
                        nc.vector._custom_dve(
                            UPD, out=tlo_new[:], in0=acc[:], in1=tau[:],
                            s0=1.0, s1=tlo[t][:], imm2=-thr)
                        tlo[t] = tlo_new
                    else:
                        ssum[t] = acc

            def emit_teardown(t):
                ot = outp.tile([P, D], f32, tag="ot", name="ot")
                nc.gpsimd.normalize_recip(ot[:], pf[t][:], ssum[t][:])
                nc.gpsimd.dma_start(out=Od[t * P:(t + 1) * P, :], in_=ot[:])

            offs = [OFF_STRIDE * t for t in range(ntiles)]
            if ntiles >= 2:
                offs[-1] -= 1  # pack the drain one step tighter
            for s in range(-1, (offs[-1] if offs else 0) + N_ITER + 2):
                for t in range(ntiles):
                    k = s - offs[t]
                    if k == -1:
                        emit_dma(t)
                    elif k == 0:
                        emit_setup(t)
                    elif 1 <= k <= N_ITER:
                        emit_pass(t, k)
                    elif k == N_ITER + 1:
                        emit_teardown(t)

    nc.finalize()
    return nc


def _get_nc(am1: float, rows: int):
    key = (am1, rows, DVE_THR, PH, OFF_STRIDE)
    if key not in _NC_CACHE:
        _NC_CACHE[key] = _build(am1, rows)
    return _NC_CACHE[key]


def _ensure_ntff_hook():
    """Register the NTFF profile hook that bass_utils needs for trace=True
    under axon (this image's antenv lacks axon_hooks; build it from the
    boot shim's ctypes driver). Also neuter the S3 artifact upload."""
    import sys as _sys
    import types

    import antenv
    import concourse.bass_utils as _bu

    _bu.upload_artifacts = lambda tmpdir: str(tmpdir)
    try:
        from antenv import axon_hooks  # noqa: F401
        return
    except ImportError:
        pass
    from trn_agent_boot.trn_boot import _ntff_profile_via_ctypes

    hook = _ntff_profile_via_ctypes("/opt/axon/libaxon_pjrt.so")
    mod = types.ModuleType("antenv.axon_hooks")
    mod._hook = hook
    mod.get_axon_ntff_profile_hook = lambda: mod._hook

    def _set(h):
        mod._hook = h

    mod.set_axon_ntff_profile_hook = _set
    _sys.modules["antenv.axon_hooks"] = mod
    antenv.axon_hooks = mod


def kernel(X, alpha):
    global LAST_RESULT
    X = np.asarray(X, dtype=np.float32)
    a = float(np.asarray(alpha, dtype=np.float32).reshape(()))
    am1 = a - 1.0
    # fast path requires am1 = 2^k so all tau/am1 rescalings are exact
    assert am1 > 0 and math.log2(am1) == round(math.log2(am1)), (
        f"unsupported alpha={a}"
    )

    orig_shape = X.shape
    Xf = np.ascontiguousarray(X.reshape(-1, D))
    rows_total = Xf.shape[0]
    assert rows_total % N_CORES == 0
    rows = rows_total // N_CORES
    shards = np.split(Xf, N_CORES, axis=0)

    nc = _get_nc(am1, rows)
    in_maps = [{"X": np.ascontiguousarray(s)} for s in shards]
    if TRACE:
        _ensure_ntff_hook()
    res = None
    for attempt in range(3):
        try:
            res = run_bass_kernel_spmd(nc, in_maps, list(range(N_CORES)),
                                       trace=TRACE)
            break
        except Exception:
            # transient NRT_EXEC_UNIT_UNRECOVERABLE happens occasionally;
            # a retry recovers the device
            if attempt == 2:
                raise
            import time
            time.sleep(5.0)
    LAST_RESULT = res
    out = np.concatenate([r["OUT"] for r in res.results], axis=0)
    return np.ascontiguousarray(out.reshape(orig_shape).astype(np.float32))
